# revision 6
# baseline (speedup 1.0000x reference)
"""Trainium2 Bass kernel for masked spatial attention softmax.

Computes S = softmax((F_a@Wq.T + bq) @ (F_s@Wk.T + bk).T / sqrt(d) + mask)
over 8 NeuronCores, data-parallel over batch.

Algebra: QK = (F_a @ Wc + bc) @ F_s.T with Wc = Wq.T @ Wk / sqrt(d) and
bc = bq @ Wk / sqrt(d) folded on the host; the bk term is constant along
the softmax axis and drops out of the softmax.  K_s is never materialized.

The mask is applied MULTIPLICATIVELY after exp (S = m*exp(QK) /
sum(m*exp(QK))), fused into a GpSimd scalar_tensor_tensor pass with
accum_out producing the masked row sums — no PE mask matmuls, no ACT
accumulator reads.  Mask tiles are broadcast across partitions by
GpSimd partition_broadcast (otherwise idle engine).

Engine budget: PE = QK + F_s/F_a transposes + one small projection;
Scalar = exp only; DVE = transpose evictions + normalize; GpSimd =
mask+rowsum; Sync = all DMA issues.
"""

import math
from contextlib import ExitStack

import numpy as np
import ml_dtypes

import concourse.bass as bass
import concourse.tile as tile
from concourse import bacc, mybir

# Problem shapes (hardcoded per contract; spec: B=32, T=256, HW=4096, d=256)
B_FULL = 32
N_CORES = 8
BS = B_FULL // N_CORES  # batches per core
T = 256
HW = 4096
D = 256
CK = 1024  # QK chunk width (2 PSUM banks)
NCK = HW // CK
SCALE = 1.0 / math.sqrt(D)  # 1/16

F32 = mybir.dt.float32
BF16 = mybir.dt.bfloat16
MUL = mybir.AluOpType.mult


def _build_body(tc, ctx, F_a, F_s, m01d, Wc, bc, S):
    nc = tc.nc

    singles = ctx.enter_context(tc.tile_pool(name="singles", bufs=1))
    fnat_pool = ctx.enter_context(tc.tile_pool(name="fnat", bufs=2))
    fst_pool = ctx.enter_context(tc.tile_pool(name="fst", bufs=2))
    qpool = ctx.enter_context(tc.tile_pool(name="qpool", bufs=2))
    m01_pool = ctx.enter_context(tc.tile_pool(name="m01", bufs=2))
    spool = ctx.enter_context(tc.tile_pool(name="spool", bufs=8))
    opool = ctx.enter_context(tc.tile_pool(name="opool", bufs=2))
    stats = ctx.enter_context(tc.tile_pool(name="stats", bufs=4))
    psum_tr = ctx.enter_context(tc.tile_pool(name="psum_tr", bufs=2, space="PSUM"))
    psum_qk = ctx.enter_context(tc.tile_pool(name="psum_qk", bufs=3, space="PSUM"))
    psum_pj = psum_tr  # projection scratch shares the transpose bank slots

    # ---- constants / prologue loads (sync queue, critical-first order) ----
    ident16 = singles.tile([128, 128], BF16, tag="ident16", name="ident16")
    ident_dram = nc.inline_tensor(
        np.eye(128, dtype=np.float32).astype(ml_dtypes.bfloat16), name="ident_c"
    )
    nc.sync.dma_start(out=ident16[:], in_=ident_dram.ap())

    # First batch's F_a first: the Q-chain (transposes) is the first PE work
    fa0 = singles.tile([128, 2, D], BF16, tag="fa0", name="fa0")
    nc.sync.dma_start(
        out=fa0[:], in_=F_a[0].rearrange("(th tl) d -> tl th d", tl=128)
    )

    wc_sb = singles.tile([128, 2, D], BF16, tag="wc", name="wc")
    nc.sync.dma_start(out=wc_sb[:], in_=Wc.rearrange("(kh kl) o -> kl kh o", kl=128))
    bc_sb = singles.tile([128, 2], F32, tag="bc", name="bc")
    nc.sync.dma_start(out=bc_sb[:], in_=bc.rearrange("(a p) -> p a", p=128))

    # F_s[0] in quarters so transposes start after the first 0.5MB
    fnat0 = fnat_pool.tile([128, 32, D], BF16, tag="fnat", name="fnat")
    fsrc0 = F_s[0].rearrange("(sh sl) c -> sl sh c", sl=128)
    for h in range(4):
        nc.sync.dma_start(
            out=fnat0[:, h * 8:(h + 1) * 8, :], in_=fsrc0[:, h * 8:(h + 1) * 8, :]
        )

    # 0/1 mask rows for all batches: [1, BS*HW] bf16
    m01_sb = singles.tile([1, BS * HW], BF16, tag="m01", name="m01")
    nc.sync.dma_start(out=m01_sb[:], in_=m01d.rearrange("b s -> (b s)")[None, :])

    fa_t, fat_t, qct_t, fnat_t, fst_t, m01_t = {}, {}, {}, {}, {}, {}
    fa_t[0] = fa0
    fnat_t[0] = fnat0

    def bcast(b):
        """Broadcast batch b's 0/1 mask row to all partitions (GpSimd)."""
        m = m01_pool.tile([128, HW], BF16, tag="m01b", name="m01b")
        for h in range(2):
            nc.gpsimd.partition_broadcast(
                m[:, h * 2048:(h + 1) * 2048],
                m01_sb[0:1, b * HW + h * 2048:b * HW + (h + 1) * 2048],
            )
        m01_t[b] = m

    def load_batch(b):
        """Prefetch F_a[b] (small, first) and F_s[b] in halves."""
        fa = qpool.tile([128, 2, D], BF16, tag="fa", name="fa")  # [tl, th, d]
        nc.sync.dma_start(
            out=fa[:], in_=F_a[b].rearrange("(th tl) d -> tl th d", tl=128)
        )
        fa_t[b] = fa
        fnat = fnat_pool.tile([128, 32, D], BF16, tag="fnat", name="fnat")
        fsrc = F_s[b].rearrange("(sh sl) c -> sl sh c", sl=128)
        for h in range(2):
            nc.sync.dma_start(
                out=fnat[:, h * 16:(h + 1) * 16, :],
                in_=fsrc[:, h * 16:(h + 1) * 16, :],
            )
        fnat_t[b] = fnat

    def qchain1(b):
        """F_a.T (PE transposes + DVE evictions)."""
        fa = fa_t.pop(b)
        fat = qpool.tile([128, 2, T], BF16, tag="fat", name="fat")  # [d_l, d_tile, t]
        for k in range(2):  # d tile
            pj = psum_pj.tile([128, T], BF16, tag="pt", name="pj")
            for m in range(2):  # t tile
                nc.tensor.matmul(
                    pj[:, m * 128:(m + 1) * 128],
                    fa[:, m, k * 128:(k + 1) * 128],
                    ident16[:],
                    is_transpose=True,
                    start=(m == 0),
                    stop=(m == 1),
                )
            nc.vector.tensor_copy(out=fat[:, k, :], in_=pj[:])
        fat_t[b] = fat

    def qchain2(b):
        """Q~.T = Wc.T @ F_a.T + bc (scale prefolded), bf16."""
        fat = fat_t.pop(b)
        qct = qpool.tile([128, 2, T], BF16, tag="qct", name="qct")
        for m in range(2):  # d_out tile
            pj = psum_pj.tile([128, T], F32, tag="pt", name="pj")
            for k in range(2):  # d_in tile
                nc.tensor.matmul(
                    pj[:],
                    wc_sb[:, k, m * 128:(m + 1) * 128],
                    fat[:, k, :],
                    start=(k == 0),
                    stop=(k == 1),
                )
            nc.vector.tensor_scalar_add(
                out=qct[:, m, :], in0=pj[:], scalar1=bc_sb[:, m:m + 1]
            )
        qct_t[b] = qct

    def qchain(b):
        qchain1(b)
        qchain2(b)

    def transpose_octet(b, ci, o):
        """8 PE transposes of [128,128] bf16 into one PSUM bank, one eviction.
        fst is split into lo/hi half-tiles so self-pair writes and same-batch
        chunk reads never touch the same tile (Tile deps are tile-granular)."""
        fnat = fnat_t[b]
        fst = fst_t[b][o // 2]
        oo = o % 2
        pt = psum_tr.tile([128, 8, 128], BF16, tag="pt", name="pt")
        for k in range(8):
            sh = o * 8 + k
            nc.tensor.matmul(
                pt[:, k, :],
                fnat[:, sh, ci * 128:(ci + 1) * 128],
                ident16[:],
                is_transpose=True,
                start=(k == 0),
                stop=(k == 7),
            )
        nc.vector.tensor_copy(
            out=fst[:, ci, oo * 1024:(oo + 1) * 1024],
            in_=pt[:].rearrange("p a b -> p (a b)"),
        )

    def qk_chunk(b, tt, ck, s_cks, st):
        """QK for one [128, 1024] chunk (2 PSUM banks), exp→bf16, then
        masked accumulate on GpSimd (in-place zeroing of masked cols)."""
        fst = fst_t[b][ck // 2]
        qct = qct_t[b]
        pq = psum_qk.tile([128, CK], F32, tag="pq", name="pq")
        # weight-reuse ordering: both banks' matmuls grouped by lhsT
        for ci in range(2):
            for h in range(2):  # 512-wide half = one PSUM bank
                s0 = (ck % 2) * 1024 + h * 512
                nc.tensor.matmul(
                    pq[:, h * 512:(h + 1) * 512],
                    qct[:, ci, tt * 128:(tt + 1) * 128],
                    fst[:, ci, s0:s0 + 512],
                    start=(ci == 0),
                    stop=(ci == 1),
                )
        s_ck = spool.tile([128, CK], BF16, tag="s", name="s")
        nc.scalar.activation(
            out=s_ck[:],
            in_=pq[:],
            func=mybir.ActivationFunctionType.Exp,
        )
        # s_ck *= m01; st[:, ck] = rowsum(masked)  (DVE: stt is DVE-only ISA)
        nc.vector.scalar_tensor_tensor(
            out=s_ck[:],
            in0=s_ck[:],
            scalar=1.0,
            in1=m01_t[b][:, ck * CK:(ck + 1) * CK],
            op0=MUL,
            op1=MUL,
            accum_out=st[:, ck:ck + 1],
        )
        s_cks.append(s_ck)

    def finish_rowtile(b, tt, s_cks, st):
        rowsum = stats.tile([128, 1], F32, tag="rowsum", name="rowsum")
        nc.vector.reduce_sum(out=rowsum[:], in_=st[:], axis=mybir.AxisListType.X)
        recip = stats.tile([128, 1], F32, tag="recip", name="recip")
        nc.vector.reciprocal(out=recip[:], in_=rowsum[:])
        o_tile = opool.tile([128, HW], BF16, tag="o", name="o")
        for ck in range(NCK):
            sl = slice(ck * CK, (ck + 1) * CK)
            # normalize on the Pool engine (keeps DVE for evictions + mask)
            nc.gpsimd.tensor_scalar_mul(
                out=o_tile[:, sl], in0=s_cks[ck][:], scalar1=recip[:, 0:1]
            )
            nc.sync.dma_start(
                out=S[b, tt * 128:(tt + 1) * 128, sl], in_=o_tile[:, sl]
            )

    # ---- software pipeline (v1-proven octet interleave) ----
    OCT0 = [(ci, o) for o in range(4) for ci in range(2)]
    fst_t[0] = (
        fst_pool.tile([128, 2, HW // 2], BF16, tag="fstlo", name="fstlo"),
        fst_pool.tile([128, 2, HW // 2], BF16, tag="fsthi", name="fsthi"),
    )
    for ci, o in OCT0[:2]:  # pair 0 -- only needs fnat quarter 0
        transpose_octet(0, ci, o)
    qchain(0)
    for ci, o in OCT0[2:4]:  # pair 1
        transpose_octet(0, ci, o)
    bcast(0)
    load_batch(1)
    qchain(1)
    bcast(1)

    for b in range(BS):
        if b + 2 < BS:
            load_batch(b + 2)
        if b + 1 < BS:
            fst_t[b + 1] = (
                fst_pool.tile([128, 2, HW // 2], BF16, tag="fstlo", name="fstlo"),
                fst_pool.tile([128, 2, HW // 2], BF16, tag="fsthi", name="fsthi"),
            )
        oi = 0
        for tt in range(2):
            s_cks = []
            st = stats.tile([128, NCK], F32, tag="st", name="st")
            for ck in range(NCK):
                if tt == 0 and ck in (0, 1):
                    # pairs 2,3 of this batch's own transposes, two chunks
                    # ahead of use
                    transpose_octet(b, *OCT0[2 * (ck + 2)])
                    transpose_octet(b, *OCT0[2 * (ck + 2) + 1])
                qk_chunk(b, tt, ck, s_cks, st)
                if b + 1 < BS and tt == 1 and oi < 4:
                    # next batch's pairs 0,1 (one octet per chunk slot)
                    transpose_octet(b + 1, *OCT0[oi])
                    oi += 1
                if b + 2 < BS:
                    # stage b+2's Q-chain + mask broadcast in free slots
                    if tt == 0 and ck == 2:
                        qchain1(b + 2)
                    elif tt == 0 and ck == 3:
                        qchain2(b + 2)
                    elif tt == 1 and ck == 0:
                        bcast(b + 2)
            finish_rowtile(b, tt, s_cks, st)
        fnat_t.pop(b, None)
        fst_t.pop(b, None)
        qct_t.pop(b, None)
        m01_t.pop(b, None)


def build_nc():
    nc = bacc.Bacc(
        "TRN2",
        target_bir_lowering=False,
        debug=False,
        num_devices=N_CORES,
    )
    F_a = nc.dram_tensor("F_a", [BS, T, D], BF16, kind="ExternalInput")
    F_s = nc.dram_tensor("F_s", [BS, HW, D], BF16, kind="ExternalInput")
    m01d = nc.dram_tensor("m01d", [BS, HW], BF16, kind="ExternalInput")
    Wc = nc.dram_tensor("Wc", [D, D], BF16, kind="ExternalInput")
    bc = nc.dram_tensor("bc", [D], F32, kind="ExternalInput")
    S = nc.dram_tensor("S", [BS, T, HW], BF16, kind="ExternalOutput")

    with tile.TileContext(nc) as tc, ExitStack() as ctx:
        _build_body(
            tc, ctx, F_a.ap(), F_s.ap(), m01d.ap(), Wc.ap(), bc.ap(), S.ap()
        )
    nc.compile()
    return nc


def make_in_maps(F_a, F_s, M_s, Wq, bq, Wk):
    F_a = np.asarray(F_a, dtype=np.float32).astype(ml_dtypes.bfloat16)
    F_s = np.asarray(F_s, dtype=np.float32).astype(ml_dtypes.bfloat16)
    M_s = np.asarray(M_s)
    Wqf = np.asarray(Wq, dtype=np.float32)
    Wkf = np.asarray(Wk, dtype=np.float32)
    bqf = np.asarray(bq, dtype=np.float32)
    # Fold: Q~ = F_a @ Wc + bc with scale pre-applied (host-side weights math)
    Wc = np.ascontiguousarray(
        ((Wqf.T @ Wkf) * np.float32(SCALE)).astype(ml_dtypes.bfloat16)
    )
    bc = np.ascontiguousarray(((bqf @ Wkf) * np.float32(SCALE)).astype(np.float32))

    m = M_s.reshape(M_s.shape[0], -1) == 1  # [B, HW]
    m01 = m.astype(np.float32).astype(ml_dtypes.bfloat16)

    in_maps = []
    for i in range(N_CORES):
        sl = slice(i * BS, (i + 1) * BS)
        in_maps.append(
            dict(
                F_a=np.ascontiguousarray(F_a[sl]),
                F_s=np.ascontiguousarray(F_s[sl]),
                m01d=np.ascontiguousarray(m01[sl]),
                Wc=Wc,
                bc=bc,
            )
        )
    return in_maps


_NC_CACHE = None


def _get_nc():
    global _NC_CACHE
    if _NC_CACHE is None:
        _NC_CACHE = build_nc()
    return _NC_CACHE


def run(in_maps, **kwargs):
    from concourse import bass_utils

    nc = _get_nc()
    res = bass_utils.run_bass_kernel_spmd(
        nc, in_maps, core_ids=list(range(N_CORES)), **kwargs
    )
    return res


def kernel(F_a, F_s, M_s, Wq, bq, Wk, bk):
    in_maps = make_in_maps(F_a, F_s, M_s, Wq, bq, Wk)
    res = run(in_maps)
    return np.concatenate(
        [np.asarray(r["S"]).astype(np.float32) for r in res.results], axis=0
    )


# revision 7
# speedup vs baseline: 5.5540x; 5.5540x over previous
"""Trainium2 Bass kernel for masked spatial attention softmax.

Computes S = softmax((F_a@Wq.T + bq) @ (F_s@Wk.T + bk).T / sqrt(d) + mask)
over 8 NeuronCores, data-parallel over batch.

Algebra: QK = (F_a @ Wc + bc) @ F_s.T with Wc = Wq.T @ Wk / sqrt(d) and
bc = bq @ Wk / sqrt(d) folded on the host; the bk term is constant along
the softmax axis and drops out of the softmax.  K_s is never materialized.

Engine assignment (fast-path ops only): PE = QK + rank-1 additive mask +
F_s/F_a transposes + one small projection; Scalar = exp (+fused accum);
DVE = transpose evictions + bias add + normalize; Sync = all DMA issues
(loads and stores), keeping Scalar's queue free of DMA work.
"""

import math
from contextlib import ExitStack

import numpy as np
import ml_dtypes

import concourse.bass as bass
import concourse.tile as tile
from concourse import bacc, mybir

# Problem shapes (hardcoded per contract; spec: B=32, T=256, HW=4096, d=256)
B_FULL = 32
N_CORES = 8
BS = B_FULL // N_CORES  # batches per core
T = 256
HW = 4096
D = 256
CK = 1024  # QK chunk width (2 PSUM banks)
NCK = HW // CK
SCALE = 1.0 / math.sqrt(D)  # 1/16
MASK_NEG = -80.0  # exp(-80 + max_logit) << 1e-30; stays in ACT exp valid range

F32 = mybir.dt.float32
BF16 = mybir.dt.bfloat16


def _build_body(tc, ctx, F_a, F_s, mbig, Wc, bc, S):
    nc = tc.nc

    singles = ctx.enter_context(tc.tile_pool(name="singles", bufs=1))
    fnat_pool = ctx.enter_context(tc.tile_pool(name="fnat", bufs=2))
    fst_pool = ctx.enter_context(tc.tile_pool(name="fst", bufs=2))
    qpool = ctx.enter_context(tc.tile_pool(name="qpool", bufs=2))
    spool = ctx.enter_context(tc.tile_pool(name="spool", bufs=8))
    opool = ctx.enter_context(tc.tile_pool(name="opool", bufs=2))
    stats = ctx.enter_context(tc.tile_pool(name="stats", bufs=4))
    psum_tr = ctx.enter_context(tc.tile_pool(name="psum_tr", bufs=2, space="PSUM"))
    psum_qk = ctx.enter_context(tc.tile_pool(name="psum_qk", bufs=3, space="PSUM"))
    psum_pj = psum_tr  # projection scratch shares the transpose bank slots

    # ---- constants / prologue loads (sync queue, critical-first order) ----
    ident16 = singles.tile([128, 128], BF16, tag="ident16", name="ident16")
    ident_dram = nc.inline_tensor(
        np.eye(128, dtype=np.float32).astype(ml_dtypes.bfloat16), name="ident_c"
    )
    nc.sync.dma_start(out=ident16[:], in_=ident_dram.ap())

    # First batch's F_a first: the Q-chain (transposes) is the first PE work
    fa0 = singles.tile([128, 2, D], BF16, tag="fa0", name="fa0")
    nc.sync.dma_start(
        out=fa0[:], in_=F_a[0].rearrange("(th tl) d -> tl th d", tl=128)
    )

    wc_sb = singles.tile([128, 2, D], BF16, tag="wc", name="wc")
    nc.sync.dma_start(out=wc_sb[:], in_=Wc.rearrange("(kh kl) o -> kl kh o", kl=128))
    bc_sb = singles.tile([128, 2], F32, tag="bc", name="bc")
    nc.sync.dma_start(out=bc_sb[:], in_=bc.rearrange("(a p) -> p a", p=128))

    ones16 = singles.tile([1, 128], BF16, tag="ones16", name="ones16")
    nc.vector.memset(ones16[:], 1.0)

    # F_s[0] in quarters so transposes start after the first 0.5MB
    fnat0 = fnat_pool.tile([128, 32, D], BF16, tag="fnat", name="fnat")
    fsrc0 = F_s[0].rearrange("(sh sl) c -> sl sh c", sl=128)
    for h in range(4):
        nc.sync.dma_start(
            out=fnat0[:, h * 8:(h + 1) * 8, :], in_=fsrc0[:, h * 8:(h + 1) * 8, :]
        )

    # additive mask rows for all batches: [1, BS*HW] bf16 (0 or MASK_NEG)
    mb_sb = singles.tile([1, BS * HW], BF16, tag="mb", name="mb")
    nc.sync.dma_start(out=mb_sb[:], in_=mbig.rearrange("b s -> (b s)")[None, :])

    fa_t, fat_t, qct_t, fnat_t, fst_t = {}, {}, {}, {}, {}
    fa_t[0] = fa0
    fnat_t[0] = fnat0

    def load_batch(b):
        """Prefetch F_a[b] (small, first) and F_s[b] in halves."""
        fa = qpool.tile([128, 2, D], BF16, tag="fa", name="fa")  # [tl, th, d]
        nc.sync.dma_start(
            out=fa[:], in_=F_a[b].rearrange("(th tl) d -> tl th d", tl=128)
        )
        fa_t[b] = fa
        fnat = fnat_pool.tile([128, 32, D], BF16, tag="fnat", name="fnat")
        fsrc = F_s[b].rearrange("(sh sl) c -> sl sh c", sl=128)
        for h in range(2):
            nc.sync.dma_start(
                out=fnat[:, h * 16:(h + 1) * 16, :],
                in_=fsrc[:, h * 16:(h + 1) * 16, :],
            )
        fnat_t[b] = fnat

    def qchain1(b):
        """F_a.T (PE transposes + DVE evictions)."""
        fa = fa_t.pop(b)
        fat = qpool.tile([128, 2, T], BF16, tag="fat", name="fat")  # [d_l, d_tile, t]
        for k in range(2):  # d tile
            pj = psum_pj.tile([128, T], BF16, tag="pt", name="pj")
            for m in range(2):  # t tile
                nc.tensor.matmul(
                    pj[:, m * 128:(m + 1) * 128],
                    fa[:, m, k * 128:(k + 1) * 128],
                    ident16[:],
                    is_transpose=True,
                    start=(m == 0),
                    stop=(m == 1),
                )
            nc.vector.tensor_copy(out=fat[:, k, :], in_=pj[:])
        fat_t[b] = fat

    def qchain2(b):
        """Q~.T = Wc.T @ F_a.T + bc (scale prefolded), bf16."""
        fat = fat_t.pop(b)
        qct = qpool.tile([128, 2, T], BF16, tag="qct", name="qct")
        for m in range(2):  # d_out tile
            pj = psum_pj.tile([128, T], F32, tag="pt", name="pj")
            for k in range(2):  # d_in tile
                nc.tensor.matmul(
                    pj[:],
                    wc_sb[:, k, m * 128:(m + 1) * 128],
                    fat[:, k, :],
                    start=(k == 0),
                    stop=(k == 1),
                )
            nc.vector.tensor_scalar_add(
                out=qct[:, m, :], in0=pj[:], scalar1=bc_sb[:, m:m + 1]
            )
        qct_t[b] = qct

    def qchain(b):
        qchain1(b)
        qchain2(b)

    def transpose_octet(b, ci, o):
        """8 PE transposes of [128,128] bf16 into one PSUM bank, one eviction.
        fst is split into lo/hi half-tiles so self-pair writes and same-batch
        chunk reads never touch the same tile (Tile deps are tile-granular)."""
        fnat = fnat_t[b]
        fst = fst_t[b][o // 2]
        oo = o % 2
        pt = psum_tr.tile([128, 8, 128], BF16, tag="pt", name="pt")
        for k in range(8):
            sh = o * 8 + k
            nc.tensor.matmul(
                pt[:, k, :],
                fnat[:, sh, ci * 128:(ci + 1) * 128],
                ident16[:],
                is_transpose=True,
                start=(k == 0),
                stop=(k == 7),
            )
        nc.vector.tensor_copy(
            out=fst[:, ci, oo * 1024:(oo + 1) * 1024],
            in_=pt[:].rearrange("p a b -> p (a b)"),
        )

    def qk_chunk(b, tt, ck, s_cks, st):
        """QK + mask for one [128, 1024] chunk (2 PSUM banks), exp→bf16 with
        fused masked-rowsum accumulation."""
        fst = fst_t[b][ck // 2]
        qct = qct_t[b]
        pq = psum_qk.tile([128, CK], F32, tag="pq", name="pq")
        # weight-reuse ordering: both banks' matmuls grouped by lhsT
        for ci in range(2):
            for h in range(2):  # 512-wide half = one PSUM bank
                s0 = (ck % 2) * 1024 + h * 512
                nc.tensor.matmul(
                    pq[:, h * 512:(h + 1) * 512],
                    qct[:, ci, tt * 128:(tt + 1) * 128],
                    fst[:, ci, s0:s0 + 512],
                    start=(ci == 0),
                    stop=False,
                )
        for h in range(2):
            mb0 = b * HW + ck * CK + h * 512
            nc.tensor.matmul(
                pq[:, h * 512:(h + 1) * 512],
                ones16[:],
                mb_sb[:, mb0:mb0 + 512],
                start=False,
                stop=True,
            )
        s_ck = spool.tile([128, CK], BF16, tag="s", name="s")
        nc.scalar.activation(
            out=s_ck[:],
            in_=pq[:],
            func=mybir.ActivationFunctionType.Exp,
            accum_out=st[:, ck:ck + 1],
        )
        s_cks.append(s_ck)

    def finish_rowtile(b, tt, s_cks, st):
        rowsum = stats.tile([128, 1], F32, tag="rowsum", name="rowsum")
        nc.vector.reduce_sum(out=rowsum[:], in_=st[:], axis=mybir.AxisListType.X)
        recip = stats.tile([128, 1], F32, tag="recip", name="recip")
        nc.vector.reciprocal(out=recip[:], in_=rowsum[:])
        o_tile = opool.tile([128, HW], BF16, tag="o", name="o")
        for ck in range(NCK):
            sl = slice(ck * CK, (ck + 1) * CK)
            nc.vector.tensor_scalar_mul(
                out=o_tile[:, sl], in0=s_cks[ck][:], scalar1=recip[:, 0:1]
            )
            nc.sync.dma_start(
                out=S[b, tt * 128:(tt + 1) * 128, sl], in_=o_tile[:, sl]
            )

    # ---- software pipeline (v1-proven octet interleave) ----
    OCT0 = [(ci, o) for o in range(4) for ci in range(2)]
    fst_t[0] = (
        fst_pool.tile([128, 2, HW // 2], BF16, tag="fstlo", name="fstlo"),
        fst_pool.tile([128, 2, HW // 2], BF16, tag="fsthi", name="fsthi"),
    )
    for ci, o in OCT0[:2]:  # pair 0 -- only needs fnat quarter 0
        transpose_octet(0, ci, o)
    qchain(0)
    for ci, o in OCT0[2:4]:  # pair 1
        transpose_octet(0, ci, o)
    load_batch(1)
    qchain(1)

    for b in range(BS):
        if b + 2 < BS:
            load_batch(b + 2)
        if b + 1 < BS:
            fst_t[b + 1] = (
                fst_pool.tile([128, 2, HW // 2], BF16, tag="fstlo", name="fstlo"),
                fst_pool.tile([128, 2, HW // 2], BF16, tag="fsthi", name="fsthi"),
            )
        oi = 0
        for tt in range(2):
            s_cks = []
            st = stats.tile([128, NCK], F32, tag="st", name="st")
            for ck in range(NCK):
                if tt == 0 and ck in (0, 1):
                    # pairs 2,3 of this batch's own transposes, two chunks
                    # ahead of use
                    transpose_octet(b, *OCT0[2 * (ck + 2)])
                    transpose_octet(b, *OCT0[2 * (ck + 2) + 1])
                qk_chunk(b, tt, ck, s_cks, st)
                if b + 1 < BS and tt == 1 and oi < 4:
                    # next batch's pairs 0,1 (one octet per chunk slot)
                    transpose_octet(b + 1, *OCT0[oi])
                    oi += 1
                if b + 2 < BS:
                    # stage b+2's Q-chain in free slots
                    if tt == 0 and ck == 2:
                        qchain1(b + 2)
                    elif tt == 0 and ck == 3:
                        qchain2(b + 2)
            finish_rowtile(b, tt, s_cks, st)
        fnat_t.pop(b, None)
        fst_t.pop(b, None)
        qct_t.pop(b, None)


def build_nc():
    nc = bacc.Bacc(
        "TRN2",
        target_bir_lowering=False,
        debug=False,
        num_devices=N_CORES,
    )
    F_a = nc.dram_tensor("F_a", [BS, T, D], BF16, kind="ExternalInput")
    F_s = nc.dram_tensor("F_s", [BS, HW, D], BF16, kind="ExternalInput")
    mbig = nc.dram_tensor("mbig", [BS, HW], BF16, kind="ExternalInput")
    Wc = nc.dram_tensor("Wc", [D, D], BF16, kind="ExternalInput")
    bc = nc.dram_tensor("bc", [D], F32, kind="ExternalInput")
    S = nc.dram_tensor("S", [BS, T, HW], BF16, kind="ExternalOutput")

    with tile.TileContext(nc) as tc, ExitStack() as ctx:
        _build_body(
            tc, ctx, F_a.ap(), F_s.ap(), mbig.ap(), Wc.ap(), bc.ap(), S.ap()
        )
    nc.compile()
    return nc


def make_in_maps(F_a, F_s, M_s, Wq, bq, Wk):
    F_a = np.asarray(F_a, dtype=np.float32).astype(ml_dtypes.bfloat16)
    F_s = np.asarray(F_s, dtype=np.float32).astype(ml_dtypes.bfloat16)
    M_s = np.asarray(M_s)
    Wqf = np.asarray(Wq, dtype=np.float32)
    Wkf = np.asarray(Wk, dtype=np.float32)
    bqf = np.asarray(bq, dtype=np.float32)
    # Fold: Q~ = F_a @ Wc + bc with scale pre-applied (host-side weights math)
    Wc = np.ascontiguousarray(
        ((Wqf.T @ Wkf) * np.float32(SCALE)).astype(ml_dtypes.bfloat16)
    )
    bc = np.ascontiguousarray(((bqf @ Wkf) * np.float32(SCALE)).astype(np.float32))

    m = M_s.reshape(M_s.shape[0], -1) == 1  # [B, HW]
    mbig = np.where(m, np.float32(0.0), np.float32(MASK_NEG)).astype(
        ml_dtypes.bfloat16
    )

    in_maps = []
    for i in range(N_CORES):
        sl = slice(i * BS, (i + 1) * BS)
        in_maps.append(
            dict(
                F_a=np.ascontiguousarray(F_a[sl]),
                F_s=np.ascontiguousarray(F_s[sl]),
                mbig=np.ascontiguousarray(mbig[sl]),
                Wc=Wc,
                bc=bc,
            )
        )
    return in_maps


_NC_CACHE = None


def _get_nc():
    global _NC_CACHE
    if _NC_CACHE is None:
        _NC_CACHE = build_nc()
    return _NC_CACHE


def run(in_maps, **kwargs):
    from concourse import bass_utils

    nc = _get_nc()
    res = bass_utils.run_bass_kernel_spmd(
        nc, in_maps, core_ids=list(range(N_CORES)), **kwargs
    )
    return res


def kernel(F_a, F_s, M_s, Wq, bq, Wk, bk):
    in_maps = make_in_maps(F_a, F_s, M_s, Wq, bq, Wk)
    res = run(in_maps)
    return np.concatenate(
        [np.asarray(r["S"]).astype(np.float32) for r in res.results], axis=0
    )
